# revision 30
# baseline (speedup 1.0000x reference)
"""Multi-head attention (B=2, S=2048, D=1024, H=16, E=64) on 8 NeuronCores.

Sharding: core c = (batch b, head-group hg) with b = c // 4, hg = c % 4.
Each core projects q/k/v for its batch into its 4 heads, runs dense
attention over the full sequence, and computes a partial output
projection with its 256 rows of Wo.  The host sums the 4 partials per
batch and adds bo (with the v-bias contribution bv @ Wo folded in).

Structure (~280us vs the 350us baseline):
  - per-head attention streams: scores/EXP for head h interleave with
    the PV accumulation of head h-1 (and with the v projection for
    h=0), so tensor and scalar engines are both >90% busy in the
    attention window.
  - scores matmuls alternate PE row quadrants per key block (via
    row-swapped bf16 copies of qhT/khT), so each LDWEIGHTS loads into
    the idle quadrant while the other streams: ~270ns per 512-col
    matmul instead of ~340.
  - PV accumulates a whole head into one [65, 2048] psum; the 65th
    psum row is the softmax denominator via a ones-column in vh
    (memset), so no separate reduction is needed.  The denominator is
    DMA-spread over 32 partitions for a wide (fast) reciprocal, then
    gathered back; all of this happens per-head inside the window.
  - EXP runs as 128 ACTs of [128, 1024] straight from PSUM (no SBUF
    staging - DVE psum reads are the scarce resource, not ACT cycles).
  - q/k biases fold into the psum eviction (tensor_scalar_add with a
    per-partition bias vector); v bias folds host-side into bo.  No
    K=1 bias matmuls.
  - x tiles are DMA'd once (dt-outer, all 4 (j, sc2) proj psums open);
    xk streams on the scalar DMA queue in parallel with xq on sync.
  - normalize of heads 0/1 and its rb broadcast overlap the last
    head's PV; output projection accumulates both j-blocks in psum and
    writes bf16 partials (summed in f32 on the host).
"""

import numpy as np

B, S, D, H, E = 2, 2048, 1024, 16, 64
HG = 4            # heads per core
N_CORES = 8
EL = E + 1        # 65: head block width in vh (values + ones column)
DT = D // 128     # 8 contraction tiles

_NC = None        # cached compiled Bass module


def _build():
    import concourse.bass as bass
    import concourse.mybir as mybir
    import concourse.tile as tile
    from concourse import bacc

    FP = mybir.dt.float32
    FPR = mybir.dt.float32r
    BF = mybir.dt.bfloat16
    F16 = mybir.dt.float16
    EXP = mybir.ActivationFunctionType.Exp

    nc = bacc.Bacc("TRN2", target_bir_lowering=False, debug=False, num_devices=1)

    xq = nc.dram_tensor("xq", [D, S], BF, kind="ExternalInput").ap()
    xk = nc.dram_tensor("xk", [D, S], BF, kind="ExternalInput").ap()
    xv = nc.dram_tensor("xv", [D, S], BF, kind="ExternalInput").ap()
    wq = nc.dram_tensor("wq", [D, HG * E], BF, kind="ExternalInput").ap()
    wk = nc.dram_tensor("wk", [D, HG * E], BF, kind="ExternalInput").ap()
    wv = nc.dram_tensor("wv", [D, HG * E], BF, kind="ExternalInput").ap()
    qkb = nc.dram_tensor("qkb", [128, 4], FP, kind="ExternalInput").ap()
    wo = nc.dram_tensor("wo", [HG * E, D], FPR, kind="ExternalInput").ap()
    eall_d = nc.dram_tensor("eall", [HG, 2, 128], FPR, kind="ExternalInput").ap()
    out = nc.dram_tensor("out_partial", [S, D], BF, kind="ExternalOutput").ap()

    with tile.TileContext(nc) as tc:
        with (
            tc.tile_pool(name="consts", bufs=1) as cpool,
            tc.tile_pool(name="resident", bufs=1) as rpool,
            tc.tile_pool(name="xqk", bufs=4) as xqkp,
            tc.tile_pool(name="xvin", bufs=2) as xvp,
            tc.tile_pool(name="exbuf", bufs=32) as expool,
            tc.tile_pool(name="stage", bufs=1) as stp,
            tc.tile_pool(name="outev", bufs=3) as osb,
        ):
            wq_sb = cpool.tile([128, DT, 256], BF, tag="wq")
            wk_sb = cpool.tile([128, DT, 256], BF, tag="wk")
            wv_sb = cpool.tile([128, DT, 256], BF, tag="wv")
            qkb_sb = cpool.tile([128, 4], FP, tag="qkb")
            eall = cpool.tile([HG, 2, 128], FPR, tag="eall")
            wo_sb = [cpool.tile([128, D], FPR, tag=f"wo{j}", name=f"wo_sb{j}") for j in range(2)]
            for dt in range(DT):
                nc.gpsimd.dma_start(wq_sb[:, dt, :], wq[dt * 128 : (dt + 1) * 128, :])
                nc.gpsimd.dma_start(wk_sb[:, dt, :], wk[dt * 128 : (dt + 1) * 128, :])
            nc.gpsimd.dma_start(qkb_sb[:], qkb[:])
            for dt in range(DT):
                nc.gpsimd.dma_start(wv_sb[:, dt, :], wv[dt * 128 : (dt + 1) * 128, :])
            nc.gpsimd.dma_start(eall[:], eall_d[:])
            for j in range(2):
                nc.gpsimd.dma_start(wo_sb[j][:], wo[j * 128 : (j + 1) * 128, :])

            qhT = rpool.tile([128, 2, S], BF, tag="qhT")
            khT = rpool.tile([128, 2, S], BF, tag="khT")
            qhTs = rpool.tile([128, 2, S], BF, tag="qhTs")   # row-halves swapped
            khTs = rpool.tile([128, 2, S], BF, tag="khTs")
            vh = rpool.tile([128, 16, HG, EL], BF, tag="vh")
            attnT = rpool.tile([128, 2, S], FPR, tag="attnT")
            sums = rpool.tile([HG, S], FPR, tag="sums")
            recip = rpool.tile([HG, S], FPR, tag="recip")
            sums_sp = rpool.tile([128, 64], FPR, tag="sums_sp")
            recip_sp = rpool.tile([128, 64], FPR, tag="recip_sp")

            # ones column of vh (softmax denominator accumulates in the
            # PV matmul); projection evictions only write cols 0:64.
            nc.vector.memset(vh[:, :, :, E : E + 1], 1.0)

            # ---- phase 1: q/k projections --------------------------------
            # dt-outer with all 4 (j, sc2) psums open: each x tile is
            # DMA'd once and read by 4 matmuls; w stationary serves 2.
            with tc.tile_pool(name="ps_proj", bufs=4, space="PSUM") as pp:
                for x_dram, w_sb, bcol, dst in (
                    (xq, wq_sb, 0, qhT),
                    (xk, wk_sb, 2, khT),
                ):
                    pss = {}
                    for j in range(2):
                        for sc2 in range(2):
                            pss[j, sc2] = pp.tile(
                                [128, 1024], FP, tag="pp", name=f"pp_{bcol}_{j}_{sc2}"
                            )
                    for dt in range(DT):
                        xt = xqkp.tile([128, S], BF, tag="xqk")
                        dma_eng = nc.sync if bcol == 0 else nc.scalar
                        dma_eng.dma_start(xt[:], x_dram[dt * 128 : (dt + 1) * 128, :])
                        for j in range(2):
                            for sc2 in range(2):
                                for hc in range(2):
                                    nc.tensor.matmul(
                                        pss[j, sc2][:, hc * 512 : (hc + 1) * 512],
                                        w_sb[:, dt, j * 128 : (j + 1) * 128],
                                        xt[:, sc2 * 1024 + hc * 512 : sc2 * 1024 + (hc + 1) * 512],
                                        start=(dt == 0),
                                        stop=(dt == DT - 1),
                                    )
                    for j in range(2):
                        for sc2 in range(2):
                            nc.vector.tensor_scalar_add(
                                dst[:, j, sc2 * 1024 : (sc2 + 1) * 1024],
                                pss[j, sc2][:],
                                qkb_sb[:, bcol + j : bcol + j + 1],
                            )
                    # row-swapped duplicate: lets scores alternate PE row
                    # quadrants per key block so LDWEIGHTS overlaps matmuls
                    dsts = qhTs if dst is qhT else khTs
                    nc.gpsimd.dma_start(dsts[0:64, :, :], dst[64:128, :, :])
                    nc.gpsimd.dma_start(dsts[64:128, :, :], dst[0:64, :, :])

            # ---- phase 2: attention (+ v projection interleaved) ---------
            exq_prev = {}   # ex tiles (tt, sc2) of the previous head
            exq_cur = {}
            pvt = [None]    # open PV psum of the previous head
            cur_sc = [None]

            def scores_unit(h, tt, scp):
                hp = h // 2
                if tt % 2 == 0:
                    kt, qt, hr = khT, qhT, (h % 2) * 64
                else:
                    kt, qt, hr = khTs, qhTs, (1 - h % 2) * 64
                for sc2 in range(2):
                    ps = scp.tile([128, 1024], FP, tag="scps", name=f"scps_{h}_{tt}_{sc2}")
                    for hc in range(2):
                        nc.tensor.matmul(
                            ps[:, hc * 512 : (hc + 1) * 512],
                            kt[hr : hr + 64, hp, tt * 128 : (tt + 1) * 128],
                            qt[hr : hr + 64, hp, sc2 * 1024 + hc * 512 : sc2 * 1024 + (hc + 1) * 512],
                            start=True,
                            stop=True,
                        )
                    exb = expool.tile([128, 1024], BF, tag="exb", name=f"exb_{h}_{tt}_{sc2}")
                    nc.scalar.activation(exb[:], ps[:], EXP, scale=0.125)
                    exq_cur[tt, sc2] = exb

            def pv_pair(h, p, pvp):
                if p == 0:
                    pvt[0] = pvp.tile([EL, S], FP, tag="pv", name=f"pv_{h}")
                for tl in range(2):
                    tt = 2 * p + tl
                    for qc in range(4):
                        sc2, hc = qc // 2, qc % 2
                        nc.tensor.matmul(
                            pvt[0][:, qc * 512 : (qc + 1) * 512],
                            vh[:, tt, h, :],
                            exq_prev[tt, sc2][:, hc * 512 : (hc + 1) * 512],
                            start=(p == 0 and tl == 0),
                            stop=(p == 7 and tl == 1),
                        )
                if p == 7:
                    hp, hr = h // 2, (h % 2) * 64
                    st = stp.tile([EL, S], FPR, tag="stage", name=f"st_{h}")
                    nc.vector.tensor_copy(st[E : E + 1, :], pvt[0][E : E + 1, :])
                    # spread the denominator over 32 partitions so the
                    # reciprocal runs wide, then gather back to [1, 2048]
                    nc.gpsimd.dma_start(sums_sp[h * 32 : (h + 1) * 32, :], st[E : E + 1, :])
                    nc.vector.tensor_copy(st[0:E, :], pvt[0][0:E, :])
                    nc.gpsimd.dma_start(attnT[hr : hr + 64, hp, :], st[0:E, :])
                    with nc.allow_low_precision(reason="fp32r recip, fp32r rb matmul"):
                        nc.vector.reciprocal(
                            recip_sp[h * 32 : (h + 1) * 32, :],
                            sums_sp[h * 32 : (h + 1) * 32, :],
                        )
                    nc.gpsimd.dma_start(recip[h : h + 1, :], recip_sp[h * 32 : (h + 1) * 32, :])

            def vproj_unit(tt, vpp, xvt_box):
                c, u = tt // 4, tt % 4
                if u == 0:
                    xvt = xvp.tile([128, DT, 512], BF, tag="xvin", name=f"xvt_{c}")
                    for dt in range(DT):
                        nc.sync.dma_start(
                            xvt[:, dt, :],
                            xv[dt * 128 : (dt + 1) * 128, c * 512 : (c + 1) * 512],
                        )
                    xvt_box[0] = xvt
                ps = vpp.tile([128, HG, E], FP, tag="ppv", name=f"ppv_{tt}")
                for dt in range(DT):
                    nc.tensor.matmul(
                        ps[:],
                        xvt_box[0][:, dt, u * 128 : (u + 1) * 128],
                        wv_sb[:, dt, :],
                        start=(dt == 0),
                        stop=(dt == DT - 1),
                    )
                nc.vector.tensor_copy(vh[:, tt, :, 0:E], ps[:])

            with tc.tile_pool(name="ps_sc", bufs=2, space="PSUM") as scp:
                xvt_box = [None]
                with tc.tile_pool(name="ps_vproj", bufs=2, space="PSUM") as vpp:
                    for tt in range(16):          # head 0 scores + v proj
                        scores_unit(0, tt, scp)
                        vproj_unit(tt, vpp, xvt_box)
                exq_prev, exq_cur = exq_cur, {}
                def norm_j(j, pool, tg):
                    # normalize via psum tiles of a pool that is free by now
                    for half in range(2):
                        rb = pool.tile([128, 1024], FP, tag=tg, name=f"rb_{j}_{half}")
                        for qc in range(2):
                            nc.tensor.matmul(
                                rb[:, qc * 512 : (qc + 1) * 512],
                                eall[:, j, :],
                                recip[:, half * 1024 + qc * 512 : half * 1024 + (qc + 1) * 512],
                                start=True,
                                stop=True,
                            )
                        sl = attnT[:, j, half * 1024 : (half + 1) * 1024]
                        nc.vector.tensor_mul(sl, sl, rb[:])

                with tc.tile_pool(name="ps_pv", bufs=1, space="PSUM") as pvp:
                    for h in range(1, HG):        # scores(h) ~ PV(h-1)
                        for tt in range(16):
                            scores_unit(h, tt, scp)
                            # PV pairs as early as possible so exbuf slots
                            # recycle before the ACT that reuses them
                            if tt < 8:
                                pv_pair(h - 1, tt, pvp)
                        exq_prev, exq_cur = exq_cur, {}
                    for p in range(8):            # PV of the last head
                        pv_pair(HG - 1, p, pvp)
                        if p == 3:
                            norm_j(0, scp, "scps")   # heads 0/1 already final

            with tc.tile_pool(name="ps_op", bufs=3, space="PSUM") as pop:
                norm_j(1, pop, "op")
                for sti in range(16):
                    op = pop.tile([128, 1024], FP, tag="op")
                    for j in range(2):
                        for hc in range(2):
                            nc.tensor.matmul(
                                op[:, hc * 512 : (hc + 1) * 512],
                                attnT[:, j, sti * 128 : (sti + 1) * 128],
                                wo_sb[j][:, hc * 512 : (hc + 1) * 512],
                                start=(j == 0),
                                stop=(j == 1),
                            )
                    ot = osb.tile([128, 1024], BF, tag="outev")
                    if sti % 2 == 0:
                        nc.vector.tensor_copy(ot[:], op[:])
                    else:
                        nc.scalar.copy(ot[:], op[:])
                    nc.sync.dma_start(out[sti * 128 : (sti + 1) * 128, :], ot[:])

    nc.compile()
    return nc


def _get_nc():
    global _NC
    if _NC is None:
        _NC = _build()
    return _NC


def _in_maps(q, k, v, Wq, bq, Wk, bk, Wv, bv, Wo, bo):
    import ml_dtypes
    f32 = np.float32
    bf16 = ml_dtypes.bfloat16

    # eall[h, j, m] = 1 iff attnT row m of j-block j belongs to head h
    eall = np.zeros((HG, 2, 128), f32)
    for h in range(HG):
        eall[h, h // 2, (h % 2) * 64 : (h % 2) * 64 + 64] = 1.0

    maps = []
    for c in range(N_CORES):
        b, hg = c // HG, c % HG
        hs = slice(hg * HG, (hg + 1) * HG)  # this core's 4 heads

        wq_h = np.transpose(Wq[hs], (1, 0, 2)).reshape(D, HG * E)
        wk_h = np.transpose(Wk[hs], (1, 0, 2)).reshape(D, HG * E)
        wv_h = np.transpose(Wv[hs], (1, 0, 2)).reshape(D, HG * E)
        qkb_h = np.stack(
            [
                bq[hs][0:2].reshape(-1),
                bq[hs][2:4].reshape(-1),
                bk[hs][0:2].reshape(-1),
                bk[hs][2:4].reshape(-1),
            ],
            axis=1,
        ).astype(f32)
        maps.append(
            {
                "xq": np.ascontiguousarray(q[b].T).astype(bf16),
                "xk": np.ascontiguousarray(k[b].T).astype(bf16),
                "xv": np.ascontiguousarray(v[b].T).astype(bf16),
                "wq": wq_h.astype(bf16),
                "wk": wk_h.astype(bf16),
                "wv": wv_h.astype(bf16),
                "qkb": qkb_h,
                "wo": np.ascontiguousarray(
                    Wo[hg * HG * E : (hg + 1) * HG * E, :], dtype=f32
                ),
                "eall": eall,
            }
        )
    return maps


def _run(inputs, trace=False):
    from concourse.bass_utils import run_bass_kernel_spmd

    nc = _get_nc()
    maps = _in_maps(**inputs)
    res = run_bass_kernel_spmd(nc, maps, list(range(N_CORES)), trace=trace)
    Wo = np.asarray(inputs["Wo"], np.float32)
    bv = np.asarray(inputs["bv"], np.float32)
    bo = np.asarray(inputs["bo"], np.float32)
    bo_eff = bo + bv.reshape(-1) @ Wo   # v bias folded through Wo
    out = np.zeros((B, S, D), np.float32)
    for b in range(B):
        acc = np.zeros((S, D), np.float32)
        for hg in range(HG):
            acc += res.results[b * HG + hg]["out_partial"].astype(np.float32)
        out[b] = acc + bo_eff[None, :]
    return out, res.exec_time_ns


def kernel(**inputs):
    out, _ = _run(inputs, trace=False)
    return out


def kernel_traced(**inputs):
    return _run(inputs, trace=True)


# revision 31
# speedup vs baseline: 1.0122x; 1.0122x over previous
"""Multi-head attention (B=2, S=2048, D=1024, H=16, E=64) on 8 NeuronCores.

Sharding: core c = (batch b, head-group hg) with b = c // 4, hg = c % 4.
Each core projects q/k/v for its batch into its 4 heads, runs dense
attention over the full sequence, and computes a partial output
projection with its 256 rows of Wo.  The host sums the 4 partials per
batch and adds bo (with the v-bias contribution bv @ Wo folded in).

Structure (~280us vs the 350us baseline):
  - per-head attention streams: scores/EXP for head h interleave with
    the PV accumulation of head h-1 (and with the v projection for
    h=0), so tensor and scalar engines are both >90% busy in the
    attention window.
  - scores matmuls alternate PE row quadrants per key block (via
    row-swapped bf16 copies of qhT/khT), so each LDWEIGHTS loads into
    the idle quadrant while the other streams: ~270ns per 512-col
    matmul instead of ~340.
  - PV accumulates a whole head into one [65, 2048] psum; the 65th
    psum row is the softmax denominator via a ones-column in vh
    (memset), so no separate reduction is needed.  The denominator is
    DMA-spread over 32 partitions for a wide (fast) reciprocal, then
    gathered back; all of this happens per-head inside the window.
  - EXP runs as 128 ACTs of [128, 1024] straight from PSUM (no SBUF
    staging - DVE psum reads are the scarce resource, not ACT cycles).
  - q/k biases fold into the psum eviction (tensor_scalar_add with a
    per-partition bias vector); v bias folds host-side into bo.  No
    K=1 bias matmuls.
  - x tiles are DMA'd once (dt-outer, all 4 (j, sc2) proj psums open);
    xk streams on the scalar DMA queue in parallel with xq on sync.
  - normalize of heads 0/1 and its rb broadcast overlap the last
    head's PV; output projection accumulates both j-blocks in psum and
    writes bf16 partials (summed in f32 on the host).
"""

import numpy as np

B, S, D, H, E = 2, 2048, 1024, 16, 64
HG = 4            # heads per core
N_CORES = 8
EL = E + 1        # 65: head block width in vh (values + ones column)
DT = D // 128     # 8 contraction tiles

_NC = None        # cached compiled Bass module


def _build():
    import concourse.bass as bass
    import concourse.mybir as mybir
    import concourse.tile as tile
    from concourse import bacc

    FP = mybir.dt.float32
    FPR = mybir.dt.float32r
    BF = mybir.dt.bfloat16
    F16 = mybir.dt.float16
    EXP = mybir.ActivationFunctionType.Exp

    nc = bacc.Bacc("TRN2", target_bir_lowering=False, debug=False, num_devices=1)

    xq = nc.dram_tensor("xq", [D, S], BF, kind="ExternalInput").ap()
    xk = nc.dram_tensor("xk", [D, S], BF, kind="ExternalInput").ap()
    xv = nc.dram_tensor("xv", [D, S], BF, kind="ExternalInput").ap()
    wq = nc.dram_tensor("wq", [D, HG * E], BF, kind="ExternalInput").ap()
    wk = nc.dram_tensor("wk", [D, HG * E], BF, kind="ExternalInput").ap()
    wv = nc.dram_tensor("wv", [D, HG * E], BF, kind="ExternalInput").ap()
    qkb = nc.dram_tensor("qkb", [128, 4], FP, kind="ExternalInput").ap()
    wo = nc.dram_tensor("wo", [HG * E, D], FPR, kind="ExternalInput").ap()
    eall_d = nc.dram_tensor("eall", [HG, 2, 128], FPR, kind="ExternalInput").ap()
    out = nc.dram_tensor("out_partial", [S, D], BF, kind="ExternalOutput").ap()

    with tile.TileContext(nc) as tc:
        with (
            tc.tile_pool(name="consts", bufs=1) as cpool,
            tc.tile_pool(name="resident", bufs=1) as rpool,
            tc.tile_pool(name="xqk", bufs=4) as xqkp,
            tc.tile_pool(name="xvin", bufs=2) as xvp,
            tc.tile_pool(name="exbuf", bufs=32) as expool,
            tc.tile_pool(name="stage", bufs=1) as stp,
            tc.tile_pool(name="outev", bufs=3) as osb,
        ):
            wq_sb = cpool.tile([128, DT, 256], BF, tag="wq")
            wk_sb = cpool.tile([128, DT, 256], BF, tag="wk")
            wv_sb = cpool.tile([128, DT, 256], BF, tag="wv")
            qkb_sb = cpool.tile([128, 4], FP, tag="qkb")
            eall = cpool.tile([HG, 2, 128], FPR, tag="eall")
            wo_sb = [cpool.tile([128, D], FPR, tag=f"wo{j}", name=f"wo_sb{j}") for j in range(2)]
            for dt in range(DT):
                nc.gpsimd.dma_start(wq_sb[:, dt, :], wq[dt * 128 : (dt + 1) * 128, :])
                nc.gpsimd.dma_start(wk_sb[:, dt, :], wk[dt * 128 : (dt + 1) * 128, :])
            nc.gpsimd.dma_start(qkb_sb[:], qkb[:])
            for dt in range(DT):
                nc.gpsimd.dma_start(wv_sb[:, dt, :], wv[dt * 128 : (dt + 1) * 128, :])
            nc.gpsimd.dma_start(eall[:], eall_d[:])
            for j in range(2):
                nc.gpsimd.dma_start(wo_sb[j][:], wo[j * 128 : (j + 1) * 128, :])

            qhT = rpool.tile([128, 2, S], BF, tag="qhT")
            khT = rpool.tile([128, 2, S], BF, tag="khT")
            qhTs = rpool.tile([128, 2, S], BF, tag="qhTs")   # row-halves swapped
            khTs = rpool.tile([128, 2, S], BF, tag="khTs")
            vh = rpool.tile([128, 16, HG, EL], BF, tag="vh")
            attnT = rpool.tile([128, 2, S], FPR, tag="attnT")
            sums = rpool.tile([HG, S], FPR, tag="sums")
            recip = rpool.tile([HG, S], FPR, tag="recip")
            sums_sp = rpool.tile([128, 64], FPR, tag="sums_sp")
            recip_sp = rpool.tile([128, 64], FPR, tag="recip_sp")

            # ones column of vh (softmax denominator accumulates in the
            # PV matmul); projection evictions only write cols 0:64.
            nc.vector.memset(vh[:, :, :, E : E + 1], 1.0)

            # ---- phase 1: q/k projections --------------------------------
            # dt-outer with all 4 (j, sc2) psums open: each x tile is
            # DMA'd once and read by 4 matmuls; w stationary serves 2.
            with tc.tile_pool(name="ps_proj", bufs=4, space="PSUM") as pp:
                for x_dram, w_sb, bcol, dst in (
                    (xq, wq_sb, 0, qhT),
                    (xk, wk_sb, 2, khT),
                ):
                    pss = {}
                    for j in range(2):
                        for sc2 in range(2):
                            pss[j, sc2] = pp.tile(
                                [128, 1024], FP, tag="pp", name=f"pp_{bcol}_{j}_{sc2}"
                            )
                    for dt in range(DT):
                        xt = xqkp.tile([128, S], BF, tag="xqk")
                        dma_eng = nc.sync if bcol == 0 else nc.scalar
                        dma_eng.dma_start(xt[:], x_dram[dt * 128 : (dt + 1) * 128, :])
                        for j in range(2):
                            for sc2 in range(2):
                                for hc in range(2):
                                    nc.tensor.matmul(
                                        pss[j, sc2][:, hc * 512 : (hc + 1) * 512],
                                        w_sb[:, dt, j * 128 : (j + 1) * 128],
                                        xt[:, sc2 * 1024 + hc * 512 : sc2 * 1024 + (hc + 1) * 512],
                                        start=(dt == 0),
                                        stop=(dt == DT - 1),
                                    )
                    for j in range(2):
                        for sc2 in range(2):
                            nc.vector.tensor_scalar_add(
                                dst[:, j, sc2 * 1024 : (sc2 + 1) * 1024],
                                pss[j, sc2][:],
                                qkb_sb[:, bcol + j : bcol + j + 1],
                            )
                    # row-swapped duplicate: lets scores alternate PE row
                    # quadrants per key block so LDWEIGHTS overlaps matmuls
                    dsts = qhTs if dst is qhT else khTs
                    nc.gpsimd.dma_start(dsts[0:64, :, :], dst[64:128, :, :])
                    nc.gpsimd.dma_start(dsts[64:128, :, :], dst[0:64, :, :])

            # ---- phase 2: attention (+ v projection interleaved) ---------
            exq_prev = {}   # ex tiles (tt, sc2) of the previous head
            exq_cur = {}
            pvt = [None]    # open PV psum of the previous head
            cur_sc = [None]

            def scores_unit(h, tt, scp):
                hp = h // 2
                if tt % 2 == 0:
                    kt, qt, hr = khT, qhT, (h % 2) * 64
                else:
                    kt, qt, hr = khTs, qhTs, (1 - h % 2) * 64
                for sc2 in range(2):
                    ps = scp.tile([128, 1024], FP, tag="scps", name=f"scps_{h}_{tt}_{sc2}")
                    for hc in range(2):
                        nc.tensor.matmul(
                            ps[:, hc * 512 : (hc + 1) * 512],
                            kt[hr : hr + 64, hp, tt * 128 : (tt + 1) * 128],
                            qt[hr : hr + 64, hp, sc2 * 1024 + hc * 512 : sc2 * 1024 + (hc + 1) * 512],
                            start=True,
                            stop=True,
                        )
                    exb = expool.tile([128, 1024], BF, tag="exb", name=f"exb_{h}_{tt}_{sc2}")
                    nc.scalar.activation(exb[:], ps[:], EXP, scale=0.125)
                    exq_cur[tt, sc2] = exb

            def pv_pair(h, p, pvp):
                if p == 0:
                    pvt[0] = pvp.tile([EL, S], FP, tag="pv", name=f"pv_{h}")
                for tl in range(2):
                    tt = 2 * p + tl
                    for qc in range(4):
                        sc2, hc = qc // 2, qc % 2
                        nc.tensor.matmul(
                            pvt[0][:, qc * 512 : (qc + 1) * 512],
                            vh[:, tt, h, :],
                            exq_prev[tt, sc2][:, hc * 512 : (hc + 1) * 512],
                            start=(p == 0 and tl == 0),
                            stop=(p == 7 and tl == 1),
                        )
                if p == 7:
                    hp, hr = h // 2, (h % 2) * 64
                    st = stp.tile([EL, S], FPR, tag="stage", name=f"st_{h}")
                    nc.vector.tensor_copy(st[E : E + 1, :], pvt[0][E : E + 1, :])
                    # spread the denominator over 32 partitions so the
                    # reciprocal runs wide, then gather back to [1, 2048]
                    nc.gpsimd.dma_start(sums_sp[h * 32 : (h + 1) * 32, :], st[E : E + 1, :])
                    if h == HG - 1:
                        # scalar engine is done with EXP by now: halve the
                        # tail-critical eviction latency
                        nc.vector.tensor_copy(st[0:E, 0:1024], pvt[0][0:E, 0:1024])
                        nc.scalar.copy(st[0:E, 1024:2048], pvt[0][0:E, 1024:2048])
                    else:
                        nc.vector.tensor_copy(st[0:E, :], pvt[0][0:E, :])
                    nc.gpsimd.dma_start(attnT[hr : hr + 64, hp, :], st[0:E, :])
                    with nc.allow_low_precision(reason="fp32r recip, fp32r rb matmul"):
                        nc.vector.reciprocal(
                            recip_sp[h * 32 : (h + 1) * 32, :],
                            sums_sp[h * 32 : (h + 1) * 32, :],
                        )
                    nc.gpsimd.dma_start(recip[h : h + 1, :], recip_sp[h * 32 : (h + 1) * 32, :])

            def vproj_unit(tt, vpp, xvt_box):
                c, u = tt // 4, tt % 4
                if u == 0:
                    xvt = xvp.tile([128, DT, 512], BF, tag="xvin", name=f"xvt_{c}")
                    for dt in range(DT):
                        nc.sync.dma_start(
                            xvt[:, dt, :],
                            xv[dt * 128 : (dt + 1) * 128, c * 512 : (c + 1) * 512],
                        )
                    xvt_box[0] = xvt
                ps = vpp.tile([128, HG, E], FP, tag="ppv", name=f"ppv_{tt}")
                for dt in range(DT):
                    nc.tensor.matmul(
                        ps[:],
                        xvt_box[0][:, dt, u * 128 : (u + 1) * 128],
                        wv_sb[:, dt, :],
                        start=(dt == 0),
                        stop=(dt == DT - 1),
                    )
                nc.vector.tensor_copy(vh[:, tt, :, 0:E], ps[:])

            with tc.tile_pool(name="ps_sc", bufs=2, space="PSUM") as scp:
                xvt_box = [None]
                with tc.tile_pool(name="ps_vproj", bufs=2, space="PSUM") as vpp:
                    for tt in range(16):          # head 0 scores + v proj
                        scores_unit(0, tt, scp)
                        vproj_unit(tt, vpp, xvt_box)
                exq_prev, exq_cur = exq_cur, {}
                def norm_j(j, pool, tg):
                    # normalize via psum tiles of a pool that is free by now
                    for half in range(2):
                        rb = pool.tile([128, 1024], FP, tag=tg, name=f"rb_{j}_{half}")
                        for qc in range(2):
                            nc.tensor.matmul(
                                rb[:, qc * 512 : (qc + 1) * 512],
                                eall[:, j, :],
                                recip[:, half * 1024 + qc * 512 : half * 1024 + (qc + 1) * 512],
                                start=True,
                                stop=True,
                            )
                        sl = attnT[:, j, half * 1024 : (half + 1) * 1024]
                        nc.vector.tensor_mul(sl, sl, rb[:])

                with tc.tile_pool(name="ps_pv", bufs=1, space="PSUM") as pvp:
                    for h in range(1, HG):        # scores(h) ~ PV(h-1)
                        for tt in range(16):
                            scores_unit(h, tt, scp)
                            # PV pairs as early as possible so exbuf slots
                            # recycle before the ACT that reuses them
                            if tt < 8:
                                pv_pair(h - 1, tt, pvp)
                        exq_prev, exq_cur = exq_cur, {}
                    for p in range(8):            # PV of the last head
                        pv_pair(HG - 1, p, pvp)
                        if p == 3:
                            norm_j(0, scp, "scps")   # heads 0/1 already final

            with tc.tile_pool(name="ps_op", bufs=3, space="PSUM") as pop:
                norm_j(1, pop, "op")
                for sti in range(16):
                    op = pop.tile([128, 1024], FP, tag="op")
                    for j in range(2):
                        for hc in range(2):
                            nc.tensor.matmul(
                                op[:, hc * 512 : (hc + 1) * 512],
                                attnT[:, j, sti * 128 : (sti + 1) * 128],
                                wo_sb[j][:, hc * 512 : (hc + 1) * 512],
                                start=(j == 0),
                                stop=(j == 1),
                            )
                    ot = osb.tile([128, 1024], BF, tag="outev")
                    nc.vector.tensor_copy(ot[:, 0:512], op[:, 0:512])
                    nc.scalar.copy(ot[:, 512:1024], op[:, 512:1024])
                    nc.sync.dma_start(
                        out[sti * 128 : (sti + 1) * 128, 0:512], ot[:, 0:512]
                    )
                    nc.sync.dma_start(
                        out[sti * 128 : (sti + 1) * 128, 512:1024], ot[:, 512:1024]
                    )

    nc.compile()
    return nc


def _get_nc():
    global _NC
    if _NC is None:
        _NC = _build()
    return _NC


def _in_maps(q, k, v, Wq, bq, Wk, bk, Wv, bv, Wo, bo):
    import ml_dtypes
    f32 = np.float32
    bf16 = ml_dtypes.bfloat16

    # eall[h, j, m] = 1 iff attnT row m of j-block j belongs to head h
    eall = np.zeros((HG, 2, 128), f32)
    for h in range(HG):
        eall[h, h // 2, (h % 2) * 64 : (h % 2) * 64 + 64] = 1.0

    maps = []
    for c in range(N_CORES):
        b, hg = c // HG, c % HG
        hs = slice(hg * HG, (hg + 1) * HG)  # this core's 4 heads

        wq_h = np.transpose(Wq[hs], (1, 0, 2)).reshape(D, HG * E)
        wk_h = np.transpose(Wk[hs], (1, 0, 2)).reshape(D, HG * E)
        wv_h = np.transpose(Wv[hs], (1, 0, 2)).reshape(D, HG * E)
        qkb_h = np.stack(
            [
                bq[hs][0:2].reshape(-1),
                bq[hs][2:4].reshape(-1),
                bk[hs][0:2].reshape(-1),
                bk[hs][2:4].reshape(-1),
            ],
            axis=1,
        ).astype(f32)
        maps.append(
            {
                "xq": np.ascontiguousarray(q[b].T).astype(bf16),
                "xk": np.ascontiguousarray(k[b].T).astype(bf16),
                "xv": np.ascontiguousarray(v[b].T).astype(bf16),
                "wq": wq_h.astype(bf16),
                "wk": wk_h.astype(bf16),
                "wv": wv_h.astype(bf16),
                "qkb": qkb_h,
                "wo": np.ascontiguousarray(
                    Wo[hg * HG * E : (hg + 1) * HG * E, :], dtype=f32
                ),
                "eall": eall,
            }
        )
    return maps


def _run(inputs, trace=False):
    from concourse.bass_utils import run_bass_kernel_spmd

    nc = _get_nc()
    maps = _in_maps(**inputs)
    res = run_bass_kernel_spmd(nc, maps, list(range(N_CORES)), trace=trace)
    Wo = np.asarray(inputs["Wo"], np.float32)
    bv = np.asarray(inputs["bv"], np.float32)
    bo = np.asarray(inputs["bo"], np.float32)
    bo_eff = bo + bv.reshape(-1) @ Wo   # v bias folded through Wo
    out = np.zeros((B, S, D), np.float32)
    for b in range(B):
        acc = np.zeros((S, D), np.float32)
        for hg in range(HG):
            acc += res.results[b * HG + hg]["out_partial"].astype(np.float32)
        out[b] = acc + bo_eff[None, :]
    return out, res.exec_time_ns


def kernel(**inputs):
    out, _ = _run(inputs, trace=False)
    return out


def kernel_traced(**inputs):
    return _run(inputs, trace=True)
